# revision 29
# baseline (speedup 1.0000x reference)
"""CRF-as-RNN mean-field kernel for Trainium2 (Bass/Tile), 8-core SPMD.

Strategy (v2):
- Shard 2 images x 4 row-strips across 8 cores; 84 rows per strip
  (64 owned + shrinking halo) so 5 mean-field iterations need no
  inter-core traffic.
- Layout: partitions = 6 row-groups x 21 channels = 126; free dim =
  14 rows x 256 cols with 2-slot halos (q: 18x260 slots).
- The 5x5 spatial gaussian (sigma=0.1) is a numerical delta: folded into
  the center tap.
- Per iteration, DVE computes ONLY the 24 bilateral tap products (fp16,
  2x mode, ~600ns per 1024-px tile); ALL accumulation runs on the PE:
  each product feeds a mneg(-compat^T) matmul accumulating straight into
  the softmax-z PSUM chunk, the center tap is a (1+wc)-scaled mneg
  matmul on q itself, and logits (fp16) enter via an identity matmul.
  No separate acc buffer, no DVE adds. (GPSIMD was tried for product
  offload: ~5x slower than the cost model claims on real HW - reverted.)
- Softmax: exp/ln on ACT, denominator via ones-mask matmul, lnD
  broadcast back into PSUM via mask matmul (exp(z-lnD), no division).
  Chunk posts are deferred one chunk-group so PE never stalls on ACT.
- Alignment: products must be 4B-aligned for DVE 2x mode on HW. A
  column-shifted copy q_odd (maintained by double-writing the softmax
  exp on ACT) plus 5 column-pre-shifted w_sc tiles (filled once by
  SBUF->SBUF DMA) make all 24 products aligned.
- Group halos are refreshed via a 21-partition-shift matmul (s_up/s_dn
  stationaries) + ACT copies from PSUM - engines cannot address
  partition base 21 directly and SBUF-SBUF DMA would serialize.
- w precompute: img in fp16; diff and square on DVE; d2 broadcast to all
  126 partitions by one mask matmul per chunk; ACT computes
  exp(-50*d2 + ln(s_k)) from PSUM straight into the w tiles (fuses the
  exp, the 21-channel broadcast, the spatial-weight scale, and the
  PSUM->SBUF copy into one ACT pass). q0 = softmax(logits) overlaps the
  precompute via a dedicated 2-buf z-PSUM ring; the last-iteration
  output is written f32 and DMA'd out per chunk.

CoreSim cost-model time: 343 us/core (baseline 866). HW differentials
(interleaved NUM_ITERS 25-vs-5): iteration cost 133 -> ~41-45 us/iter.
"""

import math
import sys
from contextlib import ExitStack

import numpy as np

sys.path.insert(0, "/opt/trn_rl_repo")

# ---------------- problem constants (hardcoded per contract) ----------------
B, C, H, W = 2, 21, 256, 256
G, RG = 6, 14                  # row groups per strip, rows per group
P = G * C                      # 126 partitions
F = RG * W                     # 3584 real pixels per partition
NT, NV = 18, 260               # q tile slots: rows -2..15, cols -2..257
WT = 16                        # w tile row slots (rows -2..13 of local strip)
IU, IV = 22, 264               # img tile slots: rows -4..17, cols -4..259
STARTS = [0, 54, 118, 172]     # strip start rows
OWN = [(0, 64), (10, 74), (10, 74), (20, 84)]  # owned local-row range per strip
NUM_ITERS = 5
NCH, CH = 7, 512               # softmax chunks (512 px = 2 rows)
CPS = [2, 2, 2, 1]             # chunk-group sizes (product granularity)
POOL_N = 0                     # tap products offloaded to GPSIMD per cp
WN = WT * NV                   # 4160 w-tile cols
PCC = 832                      # w-precompute chunk (5 chunks over 4160)

# spatial gaussian (5x5, sigma=5), normalized
_ax = np.arange(5, dtype=np.float64) - 2
_xx, _yy = np.meshgrid(_ax, _ax, indexing="ij")
_g = np.exp(-(_xx**2 + _yy**2) / (2 * 5.0**2))
SW = (_g / _g.sum()).astype(np.float64)
WC = float(SW[2, 2])           # center weight (spatial only; color=1)
# 12 unique taps (positive half-window); opposite taps share weight maps
TAPS = [(0, 1), (0, 2), (1, -2), (1, -1), (1, 0), (1, 1), (1, 2),
        (2, -2), (2, -1), (2, 0), (2, 1), (2, 2)]
ODD = [i for i, (dy, dx) in enumerate(TAPS) if dx % 2 != 0]  # need w_sc

_BASS_CACHE = {}


def _build_bass(num_iters=NUM_ITERS, skip_taps=False):
    import concourse.bass as bass
    import concourse.mybir as mybir
    from concourse import tile

    f32 = mybir.dt.float32
    f16 = mybir.dt.float16
    AF = mybir.ActivationFunctionType

    nc = bass.Bass("TRN2", target_bir_lowering=False, debug=False,
                   enable_asserts=False)

    lg_d = nc.dram_tensor("lg", [P, F], f16, kind="ExternalInput")
    img_d = nc.dram_tensor("img", [18, IU * IV], f16, kind="ExternalInput")
    mneg_d = nc.dram_tensor("mneg", [P, P], f16, kind="ExternalInput")
    mnegc_d = nc.dram_tensor("mnegc", [P, P], f16, kind="ExternalInput")
    iden_d = nc.dram_tensor("iden", [P, P], f16, kind="ExternalInput")
    onesd_d = nc.dram_tensor("onesd", [P, G], f16, kind="ExternalInput")
    bneg_d = nc.dram_tensor("bneg", [G, P], f16, kind="ExternalInput")
    bsel_d = nc.dram_tensor("bsel", [18, P], f16, kind="ExternalInput")
    bias_d = nc.dram_tensor("bias12", [P, 12], f32, kind="ExternalInput")
    sup_d = nc.dram_tensor("sup", [P, P], f16, kind="ExternalInput")
    sdn_d = nc.dram_tensor("sdn", [P, P], f16, kind="ExternalInput")
    qout_d = nc.dram_tensor("qout", [P, F], f32, kind="ExternalOutput")

    with tile.TileContext(nc) as tc, ExitStack() as ctx:
        const_pool = ctx.enter_context(tc.tile_pool(name="const", bufs=1))
        main_pool = ctx.enter_context(tc.tile_pool(name="main", bufs=1))
        w_pool = ctx.enter_context(tc.tile_pool(name="wmaps", bufs=1))

        zq_pool = ctx.enter_context(tc.tile_pool(name="zq", bufs=2,
                                                 space="PSUM"))
        hps_pool = ctx.enter_context(tc.tile_pool(name="hps", bufs=1,
                                                  space="PSUM"))
        e_pool = ctx.enter_context(tc.tile_pool(name="E", bufs=2))
        ln_pool = ctx.enter_context(tc.tile_pool(name="ln", bufs=2))

        lg_t = main_pool.tile([P, F], f16, tag="lg")
        nc.sync.dma_start(lg_t[:], lg_d.ap())
        iden_t = const_pool.tile([P, P], f16, tag="iden")
        nc.sync.dma_start(iden_t[:], iden_d.ap())
        pre_es = ExitStack()
        prep = pre_es.enter_context(tc.tile_pool(name="pre", bufs=1))
        img_t0 = prep.tile([18, IU * IV], f16, tag="img0")
        nc.scalar.dma_start(img_t0[:], img_d.ap())
        mneg_t = const_pool.tile([P, P], f16, tag="mneg")
        nc.sync.dma_start(mneg_t[:], mneg_d.ap())
        mnegc_t = const_pool.tile([P, P], f16, tag="mnegc")
        nc.sync.dma_start(mnegc_t[:], mnegc_d.ap())
        onesd_t = const_pool.tile([P, G], f16, tag="onesd")
        nc.sync.dma_start(onesd_t[:], onesd_d.ap())
        bneg_t = const_pool.tile([G, P], f16, tag="bneg")
        nc.sync.dma_start(bneg_t[:], bneg_d.ap())
        bsel_t = const_pool.tile([18, P], f16, tag="bsel")
        nc.sync.dma_start(bsel_t[:], bsel_d.ap())
        bias_t = const_pool.tile([P, 12], f32, tag="bias12")
        nc.sync.dma_start(bias_t[:], bias_d.ap())
        sup_t = const_pool.tile([P, P], f16, tag="sup")
        nc.sync.dma_start(sup_t[:], sup_d.ap())
        sdn_t = const_pool.tile([P, P], f16, tag="sdn")
        nc.sync.dma_start(sdn_t[:], sdn_d.ap())

        # Absorber matmuls: each PE matmul carries only ~1 extra sync wait,
        # so pre-observe every stationary's DMA queue with a tiny matmul.
        with tc.tile_pool(name="scrp", bufs=1, space="PSUM") as scrp:
            scr = scrp.tile([G, 2], f32, tag="scr")
            for t in (mneg_t, mnegc_t, iden_t, bneg_t, bsel_t, sup_t,
                      sdn_t):
                nc.tensor.matmul(scr[:1, :], t[:, 0:1], t[:, 0:2],
                                 start=True, stop=True, skip_group_check=True)
            nc.tensor.matmul(scr[:, :], onesd_t[:], onesd_t[:, 0:2],
                             start=True, stop=True, skip_group_check=True)

        q_t = main_pool.tile([P, NT * NV], f16, tag="q")
        q3 = q_t[:].rearrange("p (t v) -> p t v", v=NV)
        qo_t = main_pool.tile([P, NT * NV], f16, tag="qodd")
        qo3 = qo_t[:].rearrange("p (t v) -> p t v", v=NV)
        for t3 in (q3, qo3):
            nc.vector.memset(t3[:, 0:2, 0:NV], 0.0)    # top halo rows
            nc.vector.memset(t3[:, 16:18, 0:NV], 0.0)  # bottom halo rows
            nc.vector.memset(t3[:, 2:16, 0:2], 0.0)    # left halo cols
            nc.vector.memset(t3[:, 2:16, 256:NV], 0.0)  # right halo cols

        w_tiles = [w_pool.tile([P, WN], f16, tag=f"w{i}", name=f"w{i}")
                   for i in range(len(TAPS))]
        wsc_tiles = {i: w_pool.tile([P, WT * 256], f16, tag=f"wsc{i}",
                                    name=f"wsc{i}") for i in ODD}

        # softmax chunk post-processing (D-sum, lnD, broadcast, final exp).
        # `last` -> write qout instead of q; halo/odd double-writes otherwise.
        dsel = {"pool": None}

        def softmax_post(c, z_ps, last, qout3):
            e_t = e_pool.tile([P, CH], f16, tag="E")
            nc.scalar.activation(e_t[:], z_ps[:], AF.Exp)
            if dsel["pool"] is None:
                d_full = hps_pool.tile([P, CH], f32, tag="hal",
                                       name=f"dq0_{c}")
                d_ap = d_full[0:G, :]
            else:
                d_ps = dsel["pool"].tile([G, CH], f32, tag="D")
                d_ap = d_ps[:]
            nc.tensor.matmul(d_ap, onesd_t[:], e_t[:],
                             start=True, stop=True, skip_group_check=True)
            ln_t = ln_pool.tile([G, CH], f16, tag="ln")
            nc.scalar.activation(ln_t[:], d_ap, AF.Ln)
            nc.tensor.matmul(z_ps[:], bneg_t[:], ln_t[:],
                             start=False, stop=True, skip_group_check=True)
            z3 = z_ps[:].rearrange("p (r x) -> p r x", x=W)
            if last:
                nc.scalar.activation(qout3[:, 2 * c:2 * c + 2, 0:W],
                                     z3, AF.Exp)
                nc.sync.dma_start(qout_d.ap()[:, c * CH:(c + 1) * CH],
                                  qout_t[:, c * CH:(c + 1) * CH])
            else:
                r = 2 + 2 * c
                nc.scalar.activation(q3[:, r:r + 2, 2:2 + W], z3, AF.Exp)
                nc.scalar.activation(qo3[:, r:r + 2, 1:1 + W], z3, AF.Exp)
                if c == 0:   # rows 0,1 of g+1 -> lower halo slots 16,17
                    h_ps = hps_pool.tile([P, CH], f32, tag="hal",
                                         name=f"hal0_{id(z_ps)}")
                    nc.tensor.matmul(h_ps[:], sdn_t[:],
                                     q3[:, r:r + 2, 2:2 + W],
                                     start=True, stop=True,
                                     skip_group_check=True)
                    h3 = h_ps[:].rearrange("p (r x) -> p r x", x=W)
                    nc.scalar.copy(q3[:, 16:18, 2:2 + W], h3)
                    nc.scalar.copy(qo3[:, 16:18, 1:1 + W], h3)
                if c == 6:   # rows 12,13 of g-1 -> upper halo slots 0,1
                    h_ps = hps_pool.tile([P, CH], f32, tag="hal",
                                         name=f"hal6_{id(z_ps)}")
                    nc.tensor.matmul(h_ps[:], sup_t[:],
                                     q3[:, r:r + 2, 2:2 + W],
                                     start=True, stop=True,
                                     skip_group_check=True)
                    h3 = h_ps[:].rearrange("p (r x) -> p r x", x=W)
                    nc.scalar.copy(q3[:, 0:2, 2:2 + W], h3)
                    nc.scalar.copy(qo3[:, 0:2, 1:1 + W], h3)

        # ---------------- q0 = softmax(logits) ----------------
        post = None
        for c in range(NCH):
            sl = slice(c * CH, (c + 1) * CH)
            z_ps = zq_pool.tile([P, CH], f32, tag="z", name=f"zq0_{c}")
            nc.tensor.matmul(z_ps[:], iden_t[:], lg_t[:, sl],
                             start=True, stop=False, skip_group_check=True)
            if post is not None:
                softmax_post(*post, last=False, qout3=None)
            post = (c, z_ps)
        softmax_post(*post, last=False, qout3=None)

        # ---------------- w-map precompute ----------------
        with tc.tile_pool(name="sqp", bufs=2) as sqp, \
             tc.tile_pool(name="psp", bufs=2, space="PSUM") as psp:
            img3 = img_t0[:].rearrange("p (u v) -> p u v", v=IV)
            diff_t = prep.tile([18, WN], f16, tag="diff")
            diff3 = diff_t[:].rearrange("p (t v) -> p t v", v=NV)

            for ki, (dy, dx) in enumerate(TAPS):
                nc.vector.tensor_sub(
                    diff3[:, 0:WT, 0:NV],
                    img3[:, 2 + dy:2 + dy + WT, 2 + dx:2 + dx + NV],
                    img3[:, 2:2 + WT, 2:2 + NV],
                )
                sq_t = sqp.tile([18, WN], f16, tag="sq")
                nc.vector.tensor_mul(sq_t[:], diff_t[:], diff_t[:])
                for lo in range(0, WN, 1024):
                    n = min(1024, WN - lo)
                    d2_ps = psp.tile([P, 1024], f32, tag="d2")
                    for b0 in range(0, n, CH):
                        nb = min(CH, n - b0)
                        nc.tensor.matmul(d2_ps[:, b0:b0 + nb], bsel_t[:],
                                         sq_t[:, lo + b0:lo + b0 + nb],
                                         start=True, stop=True,
                                         skip_group_check=True)
                    nc.scalar.activation(w_tiles[ki][:, lo:lo + n],
                                         d2_ps[:, 0:n],
                                         AF.Exp, bias=bias_t[:, ki:ki + 1])
                if ki in ODD:
                    # w_sc[., t, u] = w[., t, u + 2 - dx]  (4B-aligned scatter
                    # reads); SBUF->SBUF DMA, off the compute engines.
                    w3 = w_tiles[ki][:].rearrange("p (t v) -> p t v", v=NV)
                    ws3 = wsc_tiles[ki][:].rearrange("p (t v) -> p t v", v=256)
                    nc.gpsimd.dma_start(ws3[:, :, 0:256],
                                        w3[:, :, 2 - dx:2 - dx + 256])

        # ---------------- iteration tiles ----------------
        pre_es.close()
        zb_pool = ctx.enter_context(tc.tile_pool(name="zb", bufs=2,
                                                 space="PSUM"))
        dps_pool = ctx.enter_context(tc.tile_pool(name="dps", bufs=2,
                                                  space="PSUM"))
        dsel["pool"] = dps_pool
        it_pool = ctx.enter_context(tc.tile_pool(name="it", bufs=1))
        qout_t = it_pool.tile([P, F], f32, tag="qout")
        qout3 = qout_t[:].rearrange("p (r x) -> p r x", x=W)
        prod_pool = ctx.enter_context(tc.tile_pool(name="prod", bufs=10))

        def product(ki, dy, dx, scatter, r0, nr, pool_eng=False):
            """DVE tap product over output rows r0..r0+nr (local, 0-based).
            Returns a dense [P, nr*256] fp16 tile (all reads 4B-aligned)."""
            t = prod_pool.tile([P, 1024], f16, tag="prod")
            t3 = t[:].rearrange("p (r x) -> p r x", x=W)
            qdy, qdx = (dy, dx) if not scatter else (-dy, -dx)
            qr = 2 + qdy + r0
            if qdx % 2 == 0:
                q_ap = q3[:, qr:qr + nr, 2 + qdx:2 + qdx + W]
            else:
                q_ap = qo3[:, qr:qr + nr, 1 + qdx:1 + qdx + W]
            if not scatter:
                w3 = w_tiles[ki][:].rearrange("p (t v) -> p t v", v=NV)
                w_ap = w3[:, 2 + r0:2 + r0 + nr, 2:2 + W]
            elif ki not in ODD:
                w3 = w_tiles[ki][:].rearrange("p (t v) -> p t v", v=NV)
                w_ap = w3[:, 2 - dy + r0:2 - dy + r0 + nr, 2 - dx:2 - dx + W]
            else:
                ws3 = wsc_tiles[ki][:].rearrange("p (t v) -> p t v", v=256)
                w_ap = ws3[:, 2 - dy + r0:2 - dy + r0 + nr, 0:W]
            eng = nc.gpsimd if pool_eng else nc.vector
            eng.tensor_mul(t3[:, 0:nr, 0:W], q_ap, w_ap)
            return t

        for it in range(num_iters):
            last = it == num_iters - 1
            pending = []
            c = 0
            for cp, nch_cp in enumerate(CPS):
                r0 = 2 * c
                nr = 2 * nch_cp
                zs = [(zq_pool if (c + j) % 2 == 0 else zb_pool).tile(
                          [P, CH], f32, tag="z", name=f"z{it}_{cp}_{j}")
                      for j in range(nch_cp)]
                first = True
                taps = TAPS if not skip_taps else []
                plist = ([(ki, dy, dx, False) for ki, (dy, dx)
                          in enumerate(taps)]
                         + [(ki, dy, dx, True) for ki, (dy, dx)
                            in enumerate(taps) if dy == 0]
                         + [(ki, dy, dx, True) for ki, (dy, dx)
                            in enumerate(taps) if dy == 1]
                         + [(ki, dy, dx, True) for ki, (dy, dx)
                            in enumerate(taps) if dy == 2])
                # last POOL_N entries run on GPSIMD, emitted first so the
                # slower engine starts early; their MMs stay in tap order.
                tiles = {}
                for ent in plist[len(plist) - POOL_N:]:
                    tiles[ent] = product(*ent, r0, nr, pool_eng=True)
                for ent in plist:
                    ki, dy, dx, sc = ent
                    t = tiles.get(ent)
                    if t is None:
                        t = product(ki, dy, dx, sc, r0, nr)
                    for j in range(nch_cp):
                        nc.tensor.matmul(
                            zs[j][:], mneg_t[:],
                            t[:, j * CH:(j + 1) * CH],
                            start=first, stop=False,
                            skip_group_check=True)
                    first = False
                for j in range(nch_cp):
                    rj = 2 + r0 + 2 * j
                    nc.tensor.matmul(
                        zs[j][:], mnegc_t[:],
                        q3[:, rj:rj + 2, 2:2 + W],
                        start=first, stop=False, skip_group_check=True)
                    nc.tensor.matmul(
                        zs[j][:], iden_t[:],
                        lg_t[:, (c + j) * CH:(c + j + 1) * CH],
                        start=False, stop=False, skip_group_check=True)
                # drain the previous chunk-pair's posts now: a full cp of
                # tap-MMs sits between a chunk's z close and its D matmul,
                # so the ACT exp is long done when PE reaches it.
                for p in pending:
                    softmax_post(*p, last=last, qout3=qout3)
                pending = [(c + j, zs[j]) for j in range(nch_cp)]
                c += nch_cp
            for p in pending:
                softmax_post(*p, last=last, qout3=qout3)


    _legalize_matmul_waits(nc, mybir)
    return nc


def _legalize_matmul_waits(nc, mybir, max_waits=2):
    """TRN2 ISA sync-wait structs hold few waits per instruction (2 for PE
    matmult/NoOp, 1 for DVE TensorTensor, ...); codegen aborts on more.
    Move excess waits onto InstNoOps (1 wait each) inserted right before
    on the same engine (adjacent => identical blocking semantics)."""
    cap = {}
    for f in nc.m.functions:
        for blk in f.blocks:
            insts = blk.instructions
            out = []
            changed = False
            for i in insts:
                si = getattr(i, "sync_info", None)
                eng = getattr(i, "engine", None)
                max_waits = cap.get(type(i).__name__, 1)
                if (si is not None and eng is not None
                        and len(si.on_wait) > max_waits):
                    waits = list(si.on_wait)
                    keep, move = [], []
                    for w in waits:
                        if "PE" in w.ant_name and len(keep) < max_waits:
                            keep.append(w)
                        else:
                            move.append(w)
                    while len(keep) < max_waits and move:
                        keep.append(move.pop())
                    nop_cap = cap.get("InstNoOp", 1)
                    while move:
                        grp, move = move[:nop_cap], move[nop_cap:]
                        nop = mybir.InstNoOp(
                            name=nc.get_next_instruction_name(),
                            engine=eng, ins=[], outs=[])
                        nop.sync_info = mybir.SyncInfo(on_wait=grp,
                                                       on_update=[])
                        out.append(nop)
                    i.sync_info = mybir.SyncInfo(
                        on_wait=keep, on_update=list(si.on_update))
                    changed = True
                out.append(i)
            if changed:
                blk.instructions = out


def _prep_shards(logits, img, compat):
    """Host-side shard prep -> list of 8 in_maps."""
    mneg = np.kron(np.eye(G), -compat.T.astype(np.float64)).astype(np.float16)
    mnegc = np.kron(np.eye(G),
                    -(1.0 + WC) * compat.T.astype(np.float64)
                    ).astype(np.float16)
    iden = np.eye(P, dtype=np.float16)
    onesd = np.kron(np.eye(G), np.ones((C, 1))).astype(np.float16)
    bneg = np.kron(np.eye(G), -np.ones((1, C))).astype(np.float16)
    bsel = np.kron(np.eye(G), -50.0 * np.ones((3, C))).astype(np.float16)
    sup = np.zeros((P, P), np.float16)
    sup[np.arange(105), np.arange(105) + 21] = 1.0   # out[i] = in[i-21]
    sdn = np.zeros((P, P), np.float16)
    sdn[np.arange(105) + 21, np.arange(105)] = 1.0   # out[i] = in[i+21]
    bias12 = np.tile(
        np.array([math.log(SW[2 + dy, 2 + dx]) for (dy, dx) in TAPS],
                 dtype=np.float32)[None, :], (P, 1))

    in_maps = []
    for core in range(8):
        b, j = divmod(core, 4)
        s = STARTS[j]
        lg = logits[b, :, s:s + 84, :].reshape(C, G, RG, W)
        lg = np.ascontiguousarray(
            lg.transpose(1, 0, 2, 3).reshape(P, F)).astype(np.float16)
        im = np.zeros((G, 3, IU, IV), np.float16)
        for g in range(G):
            base = s + g * RG - 4
            u0, u1 = max(0, -base), min(IU, H - base)
            im[g, :, u0:u1, 4:4 + W] = img[b, :, base + u0:base + u1, :]
        im = im.reshape(18, IU * IV)
        in_maps.append({
            "lg": lg, "img": np.ascontiguousarray(im),
            "mneg": mneg, "mnegc": mnegc, "iden": iden, "onesd": onesd,
            "bneg": bneg, "bsel": bsel, "bias12": bias12,
            "sup": sup, "sdn": sdn,
        })
    return in_maps


def kernel(**inputs):
    logits = np.asarray(inputs["logits"], dtype=np.float32)
    img = np.asarray(inputs["img"], dtype=np.float32)
    compat = np.asarray(inputs["compat_mat"], dtype=np.float32)

    from concourse.bass_utils import run_bass_kernel_spmd

    if "nc" not in _BASS_CACHE:
        _BASS_CACHE["nc"] = _build_bass()
    nc = _BASS_CACHE["nc"]

    in_maps = _prep_shards(logits, img, compat)
    res = run_bass_kernel_spmd(nc, in_maps, core_ids=list(range(8)))
    _BASS_CACHE["last_result"] = res

    out = np.zeros((B, C, H, W), np.float32)
    for core in range(8):
        b, j = divmod(core, 4)
        s = STARTS[j]
        lo, hi = OWN[j]
        qc = res.results[core]["qout"].reshape(G, C, RG, W)
        qc = qc.transpose(1, 0, 2, 3).reshape(C, 84, W)
        out[b, :, s + lo:s + hi, :] = qc[:, lo:hi, :]
    return out


# revision 30
# speedup vs baseline: 1.0139x; 1.0139x over previous
"""CRF-as-RNN mean-field kernel for Trainium2 (Bass/Tile), 8-core SPMD.

Strategy (v2):
- Shard 2 images x 4 row-strips across 8 cores; 84 rows per strip
  (64 owned + shrinking halo) so 5 mean-field iterations need no
  inter-core traffic.
- Layout: partitions = 6 row-groups x 21 channels = 126; free dim =
  14 rows x 256 cols with 2-slot halos (q: 18x260 slots).
- The 5x5 spatial gaussian (sigma=0.1) is a numerical delta: folded into
  the center tap.
- Per iteration, DVE computes ONLY the 24 bilateral tap products (fp16,
  2x mode, ~600ns per 1024-px tile); ALL accumulation runs on the PE:
  each product feeds a mneg(-compat^T) matmul accumulating straight into
  the softmax-z PSUM chunk, the center tap is a (1+wc)-scaled mneg
  matmul on q itself, and logits (fp16) enter via an identity matmul.
  No separate acc buffer, no DVE adds. (GPSIMD was tried for product
  offload: ~5x slower than the cost model claims on real HW - reverted.)
- Softmax: exp/ln on ACT, denominator via ones-mask matmul, lnD
  broadcast back into PSUM via mask matmul (exp(z-lnD), no division).
  Chunk posts are deferred one chunk-group so PE never stalls on ACT.
- Alignment: products must be 4B-aligned for DVE 2x mode on HW. A
  column-shifted copy q_odd (maintained by double-writing the softmax
  exp on ACT) plus 5 column-pre-shifted w_sc tiles (filled once by
  SBUF->SBUF DMA) make all 24 products aligned.
- Group halos are refreshed via a 21-partition-shift matmul (s_up/s_dn
  stationaries) + ACT copies from PSUM - engines cannot address
  partition base 21 directly and SBUF-SBUF DMA would serialize.
- w precompute: img in fp16; diff and square on DVE; d2 broadcast to all
  126 partitions by one mask matmul per chunk; ACT computes
  exp(-50*d2 + ln(s_k)) from PSUM straight into the w tiles (fuses the
  exp, the 21-channel broadcast, the spatial-weight scale, and the
  PSUM->SBUF copy into one ACT pass). q0 = softmax(logits) overlaps the
  precompute via a dedicated 2-buf z-PSUM ring; the last-iteration
  output is written f32 and DMA'd out per chunk.

CoreSim cost-model time: 343 us/core (baseline 866). HW differentials
(interleaved NUM_ITERS 25-vs-5): iteration cost 133 -> ~41-45 us/iter.
"""

import math
import sys
from contextlib import ExitStack

import numpy as np

sys.path.insert(0, "/opt/trn_rl_repo")

# ---------------- problem constants (hardcoded per contract) ----------------
B, C, H, W = 2, 21, 256, 256
G, RG = 6, 14                  # row groups per strip, rows per group
P = G * C                      # 126 partitions
F = RG * W                     # 3584 real pixels per partition
NT, NV = 18, 260               # q tile slots: rows -2..15, cols -2..257
WT = 16                        # w tile row slots (rows -2..13 of local strip)
IU, IV = 22, 264               # img tile slots: rows -4..17, cols -4..259
STARTS = [0, 54, 118, 172]     # strip start rows
OWN = [(0, 64), (10, 74), (10, 74), (20, 84)]  # owned local-row range per strip
NUM_ITERS = 5
NCH, CH = 7, 512               # softmax chunks (512 px = 2 rows)
CPS = [2, 2, 2, 1]             # chunk-group sizes (product granularity)
POOL_N = 0                     # tap products offloaded to GPSIMD per cp
WN = WT * NV                   # 4160 w-tile cols
PCC = 832                      # w-precompute chunk (5 chunks over 4160)

# spatial gaussian (5x5, sigma=5), normalized
_ax = np.arange(5, dtype=np.float64) - 2
_xx, _yy = np.meshgrid(_ax, _ax, indexing="ij")
_g = np.exp(-(_xx**2 + _yy**2) / (2 * 5.0**2))
SW = (_g / _g.sum()).astype(np.float64)
WC = float(SW[2, 2])           # center weight (spatial only; color=1)
# 12 unique taps (positive half-window); opposite taps share weight maps
TAPS = [(0, 1), (0, 2), (1, -2), (1, -1), (1, 0), (1, 1), (1, 2),
        (2, -2), (2, -1), (2, 0), (2, 1), (2, 2)]
ODD = [i for i, (dy, dx) in enumerate(TAPS) if dx % 2 != 0]  # need w_sc

_BASS_CACHE = {}


def _build_bass(num_iters=NUM_ITERS, skip_taps=False):
    import concourse.bass as bass
    import concourse.mybir as mybir
    from concourse import tile

    f32 = mybir.dt.float32
    f16 = mybir.dt.float16
    AF = mybir.ActivationFunctionType

    nc = bass.Bass("TRN2", target_bir_lowering=False, debug=False,
                   enable_asserts=False)

    lg_d = nc.dram_tensor("lg", [P, F], f16, kind="ExternalInput")
    img_d = nc.dram_tensor("img", [18, IU * IV], f16, kind="ExternalInput")
    mneg_d = nc.dram_tensor("mneg", [P, P], f16, kind="ExternalInput")
    mnegc_d = nc.dram_tensor("mnegc", [P, P], f16, kind="ExternalInput")
    iden_d = nc.dram_tensor("iden", [P, P], f16, kind="ExternalInput")
    onesd_d = nc.dram_tensor("onesd", [P, G], f16, kind="ExternalInput")
    bneg_d = nc.dram_tensor("bneg", [G, P], f16, kind="ExternalInput")
    bsel_d = nc.dram_tensor("bsel", [18, P], f16, kind="ExternalInput")
    bias_d = nc.dram_tensor("bias12", [P, 12], f32, kind="ExternalInput")
    sup_d = nc.dram_tensor("sup", [P, P], f16, kind="ExternalInput")
    sdn_d = nc.dram_tensor("sdn", [P, P], f16, kind="ExternalInput")
    qout_d = nc.dram_tensor("qout", [P, F], f32, kind="ExternalOutput")

    with tile.TileContext(nc) as tc, ExitStack() as ctx:
        const_pool = ctx.enter_context(tc.tile_pool(name="const", bufs=1))
        main_pool = ctx.enter_context(tc.tile_pool(name="main", bufs=1))
        w_pool = ctx.enter_context(tc.tile_pool(name="wmaps", bufs=1))

        zq_pool = ctx.enter_context(tc.tile_pool(name="zq", bufs=2,
                                                 space="PSUM"))
        hps_pool = ctx.enter_context(tc.tile_pool(name="hps", bufs=1,
                                                  space="PSUM"))
        e_pool = ctx.enter_context(tc.tile_pool(name="E", bufs=2))
        ln_pool = ctx.enter_context(tc.tile_pool(name="ln", bufs=2))

        lg_t = main_pool.tile([P, F], f16, tag="lg")
        nc.sync.dma_start(lg_t[:], lg_d.ap())
        iden_t = const_pool.tile([P, P], f16, tag="iden")
        nc.sync.dma_start(iden_t[:], iden_d.ap())
        pre_es = ExitStack()
        prep = pre_es.enter_context(tc.tile_pool(name="pre", bufs=1))
        img_t0 = prep.tile([18, IU * IV], f16, tag="img0")
        nc.scalar.dma_start(img_t0[:], img_d.ap())
        mneg_t = const_pool.tile([P, P], f16, tag="mneg")
        nc.sync.dma_start(mneg_t[:], mneg_d.ap())
        mnegc_t = const_pool.tile([P, P], f16, tag="mnegc")
        nc.sync.dma_start(mnegc_t[:], mnegc_d.ap())
        onesd_t = const_pool.tile([P, G], f16, tag="onesd")
        nc.sync.dma_start(onesd_t[:], onesd_d.ap())
        bneg_t = const_pool.tile([G, P], f16, tag="bneg")
        nc.sync.dma_start(bneg_t[:], bneg_d.ap())
        bsel_t = const_pool.tile([18, P], f16, tag="bsel")
        nc.sync.dma_start(bsel_t[:], bsel_d.ap())
        bias_t = const_pool.tile([P, 12], f32, tag="bias12")
        nc.sync.dma_start(bias_t[:], bias_d.ap())
        sup_t = const_pool.tile([P, P], f16, tag="sup")
        nc.sync.dma_start(sup_t[:], sup_d.ap())
        sdn_t = const_pool.tile([P, P], f16, tag="sdn")
        nc.sync.dma_start(sdn_t[:], sdn_d.ap())

        # Absorber matmuls: each PE matmul carries only ~1 extra sync wait,
        # so pre-observe every stationary's DMA queue with a tiny matmul.
        with tc.tile_pool(name="scrp", bufs=1, space="PSUM") as scrp:
            scr = scrp.tile([G, 2], f32, tag="scr")
            for t in (mneg_t, mnegc_t, iden_t, bneg_t, bsel_t, sup_t,
                      sdn_t):
                nc.tensor.matmul(scr[:1, :], t[:, 0:1], t[:, 0:2],
                                 start=True, stop=True, skip_group_check=True)
            nc.tensor.matmul(scr[:, :], onesd_t[:], onesd_t[:, 0:2],
                             start=True, stop=True, skip_group_check=True)

        q_t = main_pool.tile([P, NT * NV], f16, tag="q")
        q3 = q_t[:].rearrange("p (t v) -> p t v", v=NV)
        qo_t = main_pool.tile([P, NT * NV], f16, tag="qodd")
        qo3 = qo_t[:].rearrange("p (t v) -> p t v", v=NV)
        for t3 in (q3, qo3):
            nc.vector.memset(t3[:, 0:2, 0:NV], 0.0)    # top halo rows
            nc.vector.memset(t3[:, 16:18, 0:NV], 0.0)  # bottom halo rows
            nc.vector.memset(t3[:, 2:16, 0:2], 0.0)    # left halo cols
            nc.vector.memset(t3[:, 2:16, 256:NV], 0.0)  # right halo cols

        w_tiles = [w_pool.tile([P, WN], f16, tag=f"w{i}", name=f"w{i}")
                   for i in range(len(TAPS))]
        wsc_tiles = {i: w_pool.tile([P, WT * 256], f16, tag=f"wsc{i}",
                                    name=f"wsc{i}") for i in ODD}

        # softmax chunk post-processing (D-sum, lnD, broadcast, final exp).
        # `last` -> write qout instead of q; halo/odd double-writes otherwise.
        dsel = {"pool": None}

        def softmax_post(c, z_ps, last, qout3):
            e_t = e_pool.tile([P, CH], f16, tag="E")
            nc.scalar.activation(e_t[:], z_ps[:], AF.Exp)
            if dsel["pool"] is None:
                d_full = hps_pool.tile([P, CH], f32, tag="hal",
                                       name=f"dq0_{c}")
                d_ap = d_full[0:G, :]
            else:
                d_ps = dsel["pool"].tile([G, CH], f32, tag="D")
                d_ap = d_ps[:]
            nc.tensor.matmul(d_ap, onesd_t[:], e_t[:],
                             start=True, stop=True, skip_group_check=True)
            ln_t = ln_pool.tile([G, CH], f16, tag="ln")
            nc.scalar.activation(ln_t[:], d_ap, AF.Ln)
            nc.tensor.matmul(z_ps[:], bneg_t[:], ln_t[:],
                             start=False, stop=True, skip_group_check=True)
            z3 = z_ps[:].rearrange("p (r x) -> p r x", x=W)
            if last:
                nc.scalar.activation(qout3[:, 2 * c:2 * c + 2, 0:W],
                                     z3, AF.Exp)
                nc.sync.dma_start(qout_d.ap()[:, c * CH:(c + 1) * CH],
                                  qout_t[:, c * CH:(c + 1) * CH])
            else:
                r = 2 + 2 * c
                nc.scalar.activation(q3[:, r:r + 2, 2:2 + W], z3, AF.Exp)
                if dsel["pool"] is not None:
                    nc.scalar.activation(qo3[:, r:r + 2, 1:1 + W], z3, AF.Exp)
                if c == 0:   # rows 0,1 of g+1 -> lower halo slots 16,17
                    h_ps = hps_pool.tile([P, CH], f32, tag="hal",
                                         name=f"hal0_{id(z_ps)}")
                    nc.tensor.matmul(h_ps[:], sdn_t[:],
                                     q3[:, r:r + 2, 2:2 + W],
                                     start=True, stop=True,
                                     skip_group_check=True)
                    h3 = h_ps[:].rearrange("p (r x) -> p r x", x=W)
                    nc.scalar.copy(q3[:, 16:18, 2:2 + W], h3)
                    if dsel["pool"] is not None:
                        nc.scalar.copy(qo3[:, 16:18, 1:1 + W], h3)
                if c == 6:   # rows 12,13 of g-1 -> upper halo slots 0,1
                    h_ps = hps_pool.tile([P, CH], f32, tag="hal",
                                         name=f"hal6_{id(z_ps)}")
                    nc.tensor.matmul(h_ps[:], sup_t[:],
                                     q3[:, r:r + 2, 2:2 + W],
                                     start=True, stop=True,
                                     skip_group_check=True)
                    h3 = h_ps[:].rearrange("p (r x) -> p r x", x=W)
                    nc.scalar.copy(q3[:, 0:2, 2:2 + W], h3)
                    if dsel["pool"] is not None:
                        nc.scalar.copy(qo3[:, 0:2, 1:1 + W], h3)

        # ---------------- q0 = softmax(logits) ----------------
        post = None
        for c in range(NCH):
            sl = slice(c * CH, (c + 1) * CH)
            z_ps = zq_pool.tile([P, CH], f32, tag="z", name=f"zq0_{c}")
            nc.tensor.matmul(z_ps[:], iden_t[:], lg_t[:, sl],
                             start=True, stop=False, skip_group_check=True)
            if post is not None:
                softmax_post(*post, last=False, qout3=None)
            post = (c, z_ps)
        softmax_post(*post, last=False, qout3=None)
        # q0's q_odd in one DVE 4x copy (ACT is the precompute bottleneck)
        nc.vector.tensor_copy(qo3[:, 0:NT, 0:NV - 1], q3[:, 0:NT, 1:NV])

        # ---------------- w-map precompute ----------------
        with tc.tile_pool(name="sqp", bufs=2) as sqp, \
             tc.tile_pool(name="psp", bufs=2, space="PSUM") as psp:
            img3 = img_t0[:].rearrange("p (u v) -> p u v", v=IV)
            diff_t = prep.tile([18, WN], f16, tag="diff")
            diff3 = diff_t[:].rearrange("p (t v) -> p t v", v=NV)

            for ki, (dy, dx) in enumerate(TAPS):
                nc.vector.tensor_sub(
                    diff3[:, 0:WT, 0:NV],
                    img3[:, 2 + dy:2 + dy + WT, 2 + dx:2 + dx + NV],
                    img3[:, 2:2 + WT, 2:2 + NV],
                )
                sq_t = sqp.tile([18, WN], f16, tag="sq")
                nc.vector.tensor_mul(sq_t[:], diff_t[:], diff_t[:])
                for lo in range(0, WN, 1024):
                    n = min(1024, WN - lo)
                    d2_ps = psp.tile([P, 1024], f32, tag="d2")
                    for b0 in range(0, n, CH):
                        nb = min(CH, n - b0)
                        nc.tensor.matmul(d2_ps[:, b0:b0 + nb], bsel_t[:],
                                         sq_t[:, lo + b0:lo + b0 + nb],
                                         start=True, stop=True,
                                         skip_group_check=True)
                    nc.scalar.activation(w_tiles[ki][:, lo:lo + n],
                                         d2_ps[:, 0:n],
                                         AF.Exp, bias=bias_t[:, ki:ki + 1])
                if ki in ODD:
                    # w_sc[., t, u] = w[., t, u + 2 - dx]  (4B-aligned scatter
                    # reads); SBUF->SBUF DMA, off the compute engines.
                    w3 = w_tiles[ki][:].rearrange("p (t v) -> p t v", v=NV)
                    ws3 = wsc_tiles[ki][:].rearrange("p (t v) -> p t v", v=256)
                    nc.gpsimd.dma_start(ws3[:, :, 0:256],
                                        w3[:, :, 2 - dx:2 - dx + 256])

        # ---------------- iteration tiles ----------------
        pre_es.close()
        zb_pool = ctx.enter_context(tc.tile_pool(name="zb", bufs=2,
                                                 space="PSUM"))
        dps_pool = ctx.enter_context(tc.tile_pool(name="dps", bufs=2,
                                                  space="PSUM"))
        dsel["pool"] = dps_pool
        it_pool = ctx.enter_context(tc.tile_pool(name="it", bufs=1))
        qout_t = it_pool.tile([P, F], f32, tag="qout")
        qout3 = qout_t[:].rearrange("p (r x) -> p r x", x=W)
        prod_pool = ctx.enter_context(tc.tile_pool(name="prod", bufs=10))

        def product(ki, dy, dx, scatter, r0, nr, pool_eng=False):
            """DVE tap product over output rows r0..r0+nr (local, 0-based).
            Returns a dense [P, nr*256] fp16 tile (all reads 4B-aligned)."""
            t = prod_pool.tile([P, 1024], f16, tag="prod")
            t3 = t[:].rearrange("p (r x) -> p r x", x=W)
            qdy, qdx = (dy, dx) if not scatter else (-dy, -dx)
            qr = 2 + qdy + r0
            if qdx % 2 == 0:
                q_ap = q3[:, qr:qr + nr, 2 + qdx:2 + qdx + W]
            else:
                q_ap = qo3[:, qr:qr + nr, 1 + qdx:1 + qdx + W]
            if not scatter:
                w3 = w_tiles[ki][:].rearrange("p (t v) -> p t v", v=NV)
                w_ap = w3[:, 2 + r0:2 + r0 + nr, 2:2 + W]
            elif ki not in ODD:
                w3 = w_tiles[ki][:].rearrange("p (t v) -> p t v", v=NV)
                w_ap = w3[:, 2 - dy + r0:2 - dy + r0 + nr, 2 - dx:2 - dx + W]
            else:
                ws3 = wsc_tiles[ki][:].rearrange("p (t v) -> p t v", v=256)
                w_ap = ws3[:, 2 - dy + r0:2 - dy + r0 + nr, 0:W]
            eng = nc.gpsimd if pool_eng else nc.vector
            eng.tensor_mul(t3[:, 0:nr, 0:W], q_ap, w_ap)
            return t

        for it in range(num_iters):
            last = it == num_iters - 1
            pending = []
            c = 0
            for cp, nch_cp in enumerate(CPS):
                r0 = 2 * c
                nr = 2 * nch_cp
                zs = [(zq_pool if (c + j) % 2 == 0 else zb_pool).tile(
                          [P, CH], f32, tag="z", name=f"z{it}_{cp}_{j}")
                      for j in range(nch_cp)]
                first = True
                taps = TAPS if not skip_taps else []
                plist = ([(ki, dy, dx, False) for ki, (dy, dx)
                          in enumerate(taps)]
                         + [(ki, dy, dx, True) for ki, (dy, dx)
                            in enumerate(taps) if dy == 0]
                         + [(ki, dy, dx, True) for ki, (dy, dx)
                            in enumerate(taps) if dy == 1]
                         + [(ki, dy, dx, True) for ki, (dy, dx)
                            in enumerate(taps) if dy == 2])
                # last POOL_N entries run on GPSIMD, emitted first so the
                # slower engine starts early; their MMs stay in tap order.
                tiles = {}
                for ent in plist[len(plist) - POOL_N:]:
                    tiles[ent] = product(*ent, r0, nr, pool_eng=True)
                for ent in plist:
                    ki, dy, dx, sc = ent
                    t = tiles.get(ent)
                    if t is None:
                        t = product(ki, dy, dx, sc, r0, nr)
                    for j in range(nch_cp):
                        nc.tensor.matmul(
                            zs[j][:], mneg_t[:],
                            t[:, j * CH:(j + 1) * CH],
                            start=first, stop=False,
                            skip_group_check=True)
                    first = False
                for j in range(nch_cp):
                    rj = 2 + r0 + 2 * j
                    nc.tensor.matmul(
                        zs[j][:], mnegc_t[:],
                        q3[:, rj:rj + 2, 2:2 + W],
                        start=first, stop=False, skip_group_check=True)
                    nc.tensor.matmul(
                        zs[j][:], iden_t[:],
                        lg_t[:, (c + j) * CH:(c + j + 1) * CH],
                        start=False, stop=False, skip_group_check=True)
                # drain the previous chunk-pair's posts now: a full cp of
                # tap-MMs sits between a chunk's z close and its D matmul,
                # so the ACT exp is long done when PE reaches it.
                for p in pending:
                    softmax_post(*p, last=last, qout3=qout3)
                pending = [(c + j, zs[j]) for j in range(nch_cp)]
                c += nch_cp
            for p in pending:
                softmax_post(*p, last=last, qout3=qout3)


    _legalize_matmul_waits(nc, mybir)
    return nc


def _legalize_matmul_waits(nc, mybir, max_waits=2):
    """TRN2 ISA sync-wait structs hold few waits per instruction (2 for PE
    matmult/NoOp, 1 for DVE TensorTensor, ...); codegen aborts on more.
    Move excess waits onto InstNoOps (1 wait each) inserted right before
    on the same engine (adjacent => identical blocking semantics)."""
    cap = {}
    for f in nc.m.functions:
        for blk in f.blocks:
            insts = blk.instructions
            out = []
            changed = False
            for i in insts:
                si = getattr(i, "sync_info", None)
                eng = getattr(i, "engine", None)
                max_waits = cap.get(type(i).__name__, 1)
                if (si is not None and eng is not None
                        and len(si.on_wait) > max_waits):
                    waits = list(si.on_wait)
                    keep, move = [], []
                    for w in waits:
                        if "PE" in w.ant_name and len(keep) < max_waits:
                            keep.append(w)
                        else:
                            move.append(w)
                    while len(keep) < max_waits and move:
                        keep.append(move.pop())
                    nop_cap = cap.get("InstNoOp", 1)
                    while move:
                        grp, move = move[:nop_cap], move[nop_cap:]
                        nop = mybir.InstNoOp(
                            name=nc.get_next_instruction_name(),
                            engine=eng, ins=[], outs=[])
                        nop.sync_info = mybir.SyncInfo(on_wait=grp,
                                                       on_update=[])
                        out.append(nop)
                    i.sync_info = mybir.SyncInfo(
                        on_wait=keep, on_update=list(si.on_update))
                    changed = True
                out.append(i)
            if changed:
                blk.instructions = out


def _prep_shards(logits, img, compat):
    """Host-side shard prep -> list of 8 in_maps."""
    mneg = np.kron(np.eye(G), -compat.T.astype(np.float64)).astype(np.float16)
    mnegc = np.kron(np.eye(G),
                    -(1.0 + WC) * compat.T.astype(np.float64)
                    ).astype(np.float16)
    iden = np.eye(P, dtype=np.float16)
    onesd = np.kron(np.eye(G), np.ones((C, 1))).astype(np.float16)
    bneg = np.kron(np.eye(G), -np.ones((1, C))).astype(np.float16)
    bsel = np.kron(np.eye(G), -50.0 * np.ones((3, C))).astype(np.float16)
    sup = np.zeros((P, P), np.float16)
    sup[np.arange(105), np.arange(105) + 21] = 1.0   # out[i] = in[i-21]
    sdn = np.zeros((P, P), np.float16)
    sdn[np.arange(105) + 21, np.arange(105)] = 1.0   # out[i] = in[i+21]
    bias12 = np.tile(
        np.array([math.log(SW[2 + dy, 2 + dx]) for (dy, dx) in TAPS],
                 dtype=np.float32)[None, :], (P, 1))

    in_maps = []
    for core in range(8):
        b, j = divmod(core, 4)
        s = STARTS[j]
        lg = logits[b, :, s:s + 84, :].reshape(C, G, RG, W)
        lg = np.ascontiguousarray(
            lg.transpose(1, 0, 2, 3).reshape(P, F)).astype(np.float16)
        im = np.zeros((G, 3, IU, IV), np.float16)
        for g in range(G):
            base = s + g * RG - 4
            u0, u1 = max(0, -base), min(IU, H - base)
            im[g, :, u0:u1, 4:4 + W] = img[b, :, base + u0:base + u1, :]
        im = im.reshape(18, IU * IV)
        in_maps.append({
            "lg": lg, "img": np.ascontiguousarray(im),
            "mneg": mneg, "mnegc": mnegc, "iden": iden, "onesd": onesd,
            "bneg": bneg, "bsel": bsel, "bias12": bias12,
            "sup": sup, "sdn": sdn,
        })
    return in_maps


def kernel(**inputs):
    logits = np.asarray(inputs["logits"], dtype=np.float32)
    img = np.asarray(inputs["img"], dtype=np.float32)
    compat = np.asarray(inputs["compat_mat"], dtype=np.float32)

    from concourse.bass_utils import run_bass_kernel_spmd

    if "nc" not in _BASS_CACHE:
        _BASS_CACHE["nc"] = _build_bass()
    nc = _BASS_CACHE["nc"]

    in_maps = _prep_shards(logits, img, compat)
    res = run_bass_kernel_spmd(nc, in_maps, core_ids=list(range(8)))
    _BASS_CACHE["last_result"] = res

    out = np.zeros((B, C, H, W), np.float32)
    for core in range(8):
        b, j = divmod(core, 4)
        s = STARTS[j]
        lo, hi = OWN[j]
        qc = res.results[core]["qout"].reshape(G, C, RG, W)
        qc = qc.transpose(1, 0, 2, 3).reshape(C, 84, W)
        out[b, :, s + lo:s + hi, :] = qc[:, lo:hi, :]
    return out
